# revision 9
# baseline (speedup 1.0000x reference)
"""Causal self-attention (B=4, T=2048, C=1024, H=16) on 8 trn2 NeuronCores.

Sharding: core c -> (batch b = c//2, head-group g = c%2 of 8 heads).
Each core computes its batch's QKV for its 8 heads, causal attention,
and a partial output projection (its heads' rows of w_out). Host sums
the two partials per batch and adds b_out.
"""
import sys
sys.path.insert(0, "/opt/trn_rl_repo")

import numpy as np
import concourse.bass as bass
import concourse.mybir as mybir
import concourse.tile as tile
from concourse import bacc
from concourse.bass_utils import run_bass_kernel_spmd
from concourse.tile import TileContext

F32 = mybir.dt.float32
F32R = mybir.dt.float32r
BF16 = mybir.dt.bfloat16
AF = mybir.ActivationFunctionType

B, T, C = 4, 2048, 1024
H, D = 16, 64
HL = 8            # heads per core
PAIRS = HL // 2   # head pairs (128-partition stacking)
KCH = C // 128    # contraction chunks for QKV
TG = T // 512     # 512-wide token groups
NKT = T // 128    # 128-wide key tiles
SCALE = D ** -0.5

_cache = {}


def _build():
    nc = bacc.Bacc("TRN2", target_bir_lowering=False, debug=False, num_devices=8)

    xt_d = nc.dram_tensor("xt", [C, T], F32R, kind="ExternalInput")
    wqk_d = nc.dram_tensor("wqk", [C, 1024], F32R, kind="ExternalInput")
    wv_d = nc.dram_tensor("wv", [C + 1, 512], F32R, kind="ExternalInput")
    bqk_d = nc.dram_tensor("bqk", [128, 8], F32, kind="ExternalInput")
    wo_d = nc.dram_tensor("wo", [512, 1024], F32R, kind="ExternalInput")
    y_d = nc.dram_tensor("y", [T, C], F32, kind="ExternalOutput")

    with TileContext(nc) as tc:
        with tc.tile_pool(name="persist", bufs=1) as persist:
            qkT = persist.tile([128, 8, T], F32R)          # tiles 0-3: q pairs, 4-7: k pairs
            v_aug = persist.tile([128, NKT, HL, D + 1], BF16)
            attout = persist.tile([128, PAIRS, T], F32R)
            bqk_sb = persist.tile([128, 8], F32)
            ones1 = persist.tile([1, 128], F32R)
            ones1_f = persist.tile([1, 128], F32)

            nc.sync.dma_start(out=bqk_sb, in_=bqk_d[:])
            nc.vector.memset(ones1_f, 1.0)
            nc.vector.tensor_copy(ones1, ones1_f)
            nc.vector.memset(v_aug[:, :, :, D:D + 1], 1.0)

            # ---------------- Phase 1: QKV ----------------
            with tc.tile_pool(name="qkvw", bufs=1) as wpool, \
                 tc.tile_pool(name="xts", bufs=10) as xpool, \
                 tc.tile_pool(name="qk_ps", bufs=4, space="PSUM") as qk_psum, \
                 tc.tile_pool(name="v_ps", bufs=2, space="PSUM") as v_psum:
                wqk_sb = wpool.tile([128, KCH, 1024], F32R)
                wv_sb = wpool.tile([128, KCH, 512], F32R)
                wv_last = wpool.tile([1, 512], F32R)
                nc.sync.dma_start(
                    out=wqk_sb, in_=wqk_d.rearrange("(k p) c -> p k c", p=128))
                nc.sync.dma_start(
                    out=wv_sb, in_=wv_d[0:C, :].rearrange("(k p) c -> p k c", p=128))
                nc.sync.dma_start(out=wv_last, in_=wv_d[C:C + 1, :])

                xt_r = xt_d.rearrange("(k p) t -> p k t", p=128)
                for gi in range(TG):
                    xts = []
                    for k in range(KCH):
                        xk = xpool.tile([128, 512], F32R, tag="x")
                        nc.sync.dma_start(
                            out=xk, in_=xt_r[:, k, 512 * gi:512 * (gi + 1)])
                        xts.append(xk)
                    for t in range(8):
                        ps = qk_psum.tile([128, 512], F32)
                        for k in range(KCH):
                            nc.tensor.matmul(
                                ps, wqk_sb[:, k, 128 * t:128 * (t + 1)], xts[k],
                                start=(k == 0), stop=(k == KCH - 1))
                        nc.vector.tensor_scalar_add(
                            qkT[:, t, 512 * gi:512 * (gi + 1)], ps,
                            bqk_sb[:, t:t + 1])
                    for tt in range(4):
                        tau = 4 * gi + tt
                        ps = v_psum.tile([128, 512], F32)
                        for k in range(KCH):
                            nc.tensor.matmul(
                                ps, xts[k][:, 128 * tt:128 * (tt + 1)],
                                wv_sb[:, k, :], start=(k == 0), stop=False)
                        nc.tensor.matmul(
                            ps, ones1, wv_last, start=False, stop=True)
                        nc.vector.tensor_copy(
                            v_aug[:, tau, :, 0:D],
                            ps.rearrange("p (h d) -> p h d", h=HL))

            # ---------------- Phase 2: attention ----------------
            with tc.tile_pool(name="att", bufs=2) as att_pool, \
                 tc.tile_pool(name="nrm", bufs=2) as nrm_pool, \
                 tc.tile_pool(name="sc_ps", bufs=1, space="PSUM") as sc_psum, \
                 tc.tile_pool(name="av_ps", bufs=2, space="PSUM") as av_psum, \
                 tc.tile_pool(name="map_ps", bufs=2, space="PSUM") as map_psum:
                for h in range(HL):
                    p, r = h // 2, 64 * (h % 2)
                    q_t = qkT[r:r + 64, p, :]       # [64, T] f32r
                    k_t = qkT[r:r + 64, 4 + p, :]   # [64, T]

                    # scores S^T[k_tok, q] + exp -> bf16 att tiles
                    atts = []
                    for ki in range(NKT):
                        qlo = 128 * ki
                        sc = sc_psum.tile([128, T], F32)
                        lhsT = k_t[:, qlo:qlo + 128]
                        qc = qlo
                        while qc < T:
                            qe = min(512 * (qc // 512 + 1), T)
                            nc.tensor.matmul(
                                sc[:, qc:qe], lhsT, q_t[:, qc:qe],
                                start=True, stop=True)
                            qc = qe
                        at = att_pool.tile([128, T - qlo], BF16, tag=f"att{ki}")
                        nc.scalar.activation(at, sc[:, qlo:T], AF.Exp, scale=SCALE)
                        # zero strictly-upper part of the diagonal 128x128 block
                        nc.gpsimd.affine_select(
                            out=at[:, 0:128], in_=at[:, 0:128],
                            compare_op=mybir.AluOpType.is_ge, fill=0.0,
                            base=0, pattern=[[1, 128]], channel_multiplier=-1)
                        atts.append(at)

                    # AV: out_aug[65, q] accumulated over ki; row 64 = denom
                    for gi in range(TG):
                        av = av_psum.tile([128, 512], F32)
                        for ki in range(min(4 * gi + 4, NKT)):
                            qlo = 128 * ki
                            g0 = 512 * gi
                            lo = max(g0, qlo)
                            nc.tensor.matmul(
                                av[0:65, lo - g0:512],
                                v_aug[:, ki, h, :],
                                atts[ki][:, lo - qlo:512 * (gi + 1) - qlo],
                                start=(ki == 0), stop=(ki == min(4 * gi + 3, NKT - 1)))
                        # normalize: attout[r:r+64] = av[0:64] / bcast(av[64])
                        den = nrm_pool.tile([1, 512], F32R, tag="den")
                        nc.vector.tensor_copy(den, av[64:65, :])
                        mp = map_psum.tile([64, 512], F32)
                        nc.tensor.matmul(mp, ones1[:, 0:64], den,
                                         start=True, stop=True)
                        rmap = nrm_pool.tile([64, 512], F32, tag="rmap")
                        nc.vector.reciprocal(rmap, mp)
                        nc.vector.tensor_mul(
                            attout[r:r + 64, p, 512 * gi:512 * (gi + 1)],
                            av[0:64, :], rmap)

            # ---------------- Phase 3: output projection ----------------
            with tc.tile_pool(name="proj", bufs=1) as wopool, \
                 tc.tile_pool(name="ysb", bufs=4) as ypool, \
                 tc.tile_pool(name="y_ps", bufs=4, space="PSUM") as y_psum:
                wo_sb = wopool.tile([128, PAIRS, 1024], F32R)
                nc.sync.dma_start(
                    out=wo_sb, in_=wo_d.rearrange("(p c) e -> c p e", c=128))
                for tau in range(NKT):
                    for eg in range(2):
                        ps = y_psum.tile([128, 512], F32)
                        for p in range(PAIRS):
                            nc.tensor.matmul(
                                ps, attout[:, p, 128 * tau:128 * (tau + 1)],
                                wo_sb[:, p, 512 * eg:512 * (eg + 1)],
                                start=(p == 0), stop=(p == PAIRS - 1))
                        ysb = ypool.tile([128, 512], F32)
                        nc.vector.tensor_copy(ysb, ps)
                        nc.sync.dma_start(
                            out=y_d[128 * tau:128 * (tau + 1),
                                    512 * eg:512 * (eg + 1)],
                            in_=ysb)

    nc.compile()
    return nc


def _prep_inputs(x, w_qkv, b_qkv, w_out, b_out):
    x = np.asarray(x, np.float32)
    w_qkv = np.asarray(w_qkv, np.float32)
    b_qkv = np.asarray(b_qkv, np.float32)
    w_out = np.asarray(w_out, np.float32)
    in_maps = []
    for c in range(8):
        b, g = c // 2, c % 2
        xt = np.ascontiguousarray(x[b].T)
        wqk = np.concatenate(
            [w_qkv[:, 512 * g:512 * g + 512],
             w_qkv[:, C + 512 * g:C + 512 * g + 512]], axis=1)
        bqk = np.concatenate(
            [b_qkv[512 * g:512 * g + 512],
             b_qkv[C + 512 * g:C + 512 * g + 512]]).reshape(8, 128).T
        wv = np.concatenate(
            [w_qkv[:, 2 * C + 512 * g:2 * C + 512 * g + 512],
             b_qkv[None, 2 * C + 512 * g:2 * C + 512 * g + 512]], axis=0)
        wo = w_out[512 * g:512 * g + 512, :]
        in_maps.append({
            "xt": np.ascontiguousarray(xt),
            "wqk": np.ascontiguousarray(wqk),
            "bqk": np.ascontiguousarray(bqk),
            "wv": np.ascontiguousarray(wv),
            "wo": np.ascontiguousarray(wo),
        })
    return in_maps


def kernel(x, w_qkv, b_qkv, w_out, b_out):
    if "nc" not in _cache:
        _cache["nc"] = _build()
    nc = _cache["nc"]
    in_maps = _prep_inputs(x, w_qkv, b_qkv, w_out, b_out)
    res = run_bass_kernel_spmd(nc, in_maps, list(range(8)))
    b_out = np.asarray(b_out, np.float32)
    out = np.empty((B, T, C), np.float32)
    for b in range(B):
        out[b] = res.results[2 * b]["y"] + res.results[2 * b + 1]["y"] + b_out
    return out


def bench(x, w_qkv, b_qkv, w_out, b_out, iters=16, reps=3):
    """Time the NEFF by chaining `iters` executions inside one jitted call
    (consecutive executions serialized via a zero-valued data dependency),
    subtracting a 1-iter call to remove dispatch/transfer overhead.
    Returns per-execution seconds (min over reps)."""
    import time
    import jax
    import jax.numpy as jnp
    from jax.sharding import Mesh, PartitionSpec
    from jax.experimental.shard_map import shard_map
    from concourse import bass2jax
    from concourse.bass2jax import (
        _bass_exec_p, install_neuronx_cc_hook, partition_id_tensor)

    if "nc" not in _cache:
        _cache["nc"] = _build()
    nc = _cache["nc"]
    install_neuronx_cc_hook()
    in_maps = _prep_inputs(x, w_qkv, b_qkv, w_out, b_out)

    partition_name = (nc.partition_id_tensor.name
                      if nc.partition_id_tensor else None)
    in_names, out_names, out_avals, zero_outs = [], [], [], []
    for alloc in nc.m.functions[0].allocations:
        if not isinstance(alloc, mybir.MemoryLocationSet):
            continue
        name = alloc.memorylocations[0].name
        if alloc.kind == "ExternalInput":
            if name != partition_name:
                in_names.append(name)
        elif alloc.kind == "ExternalOutput":
            out_names.append(name)
            shape = tuple(alloc.tensor_shape)
            dtype = mybir.dt.np(alloc.dtype)
            out_avals.append(jax.core.ShapedArray(shape, dtype))
            zero_outs.append(np.zeros(shape, dtype))
    n_params = len(in_names)
    all_names = in_names + out_names
    if partition_name is not None:
        all_names.append(partition_name)
    chain_idx = in_names.index("bqk")

    def body_n(n):
        def _body(*args):
            ins = list(args)
            outs = None
            for _ in range(n):
                cur = list(ins)
                if outs is not None:
                    y = outs[0]
                    cur[chain_idx] = cur[chain_idx] + 0.0 * y[:128, :8]
                if partition_name is not None:
                    cur.append(partition_id_tensor())
                outs = _bass_exec_p.bind(
                    *cur,
                    out_avals=tuple(out_avals),
                    in_names=tuple(all_names),
                    out_names=tuple(out_names),
                    lowering_input_output_aliases=(),
                    sim_require_finite=True,
                    sim_require_nnan=True,
                    nc=nc,
                )
            return tuple(outs)
        return _body

    devices = jax.devices()[:8]
    mesh = Mesh(np.asarray(devices), ("core",))
    in_specs = (PartitionSpec("core"),) * (n_params + len(out_names))
    out_specs = (PartitionSpec("core"),) * len(out_names)

    per_core = [[np.asarray(m[name]) for name in in_names] for m in in_maps]
    concat_in = [np.concatenate([per_core[c][i] for c in range(8)], axis=0)
                 for i in range(n_params)]
    concat_zero = [np.zeros((8 * z.shape[0], *z.shape[1:]), z.dtype)
                   for z in zero_outs]
    ins_dev = [jax.device_put(a) for a in concat_in]
    donate = tuple(range(n_params, n_params + len(zero_outs)))

    f = jax.jit(shard_map(body_n(1), mesh=mesh, in_specs=in_specs,
                          out_specs=out_specs, check_rep=False),
                donate_argnums=donate, keep_unused=True)

    def fresh_zeros(n):
        return [[jax.device_put(z) for z in concat_zero] for _ in range(n)]

    z0 = fresh_zeros(1)[0]
    jax.block_until_ready(f(*ins_dev, *z0))  # compile + warm

    def timed(n):
        best = float("inf")
        for _ in range(reps):
            zs = fresh_zeros(n)
            jax.block_until_ready(zs)
            t0 = time.perf_counter()
            rs = [f(*ins_dev, *zs[i]) for i in range(n)]
            jax.block_until_ready(rs)
            best = min(best, time.perf_counter() - t0)
        return best

    n_lo, n_hi = max(1, iters // 4), iters
    t_lo = timed(n_lo)
    t_hi = timed(n_hi)
    return (t_hi - t_lo) / (n_hi - n_lo), t_lo, t_hi
